# revision 14
# baseline (speedup 1.0000x reference)
"""Adaptive average pooling [8,224,224,256] -> [8,7,7,256] on 8 TRN2 NeuronCores.

Strategy: data-parallel over batch (1 sample per core, no collectives).
Pooling windows are exact 32x32 blocks (224/7 = 32). Each sample is
repacked host-side (channel-group-major, w innermost) and cast to fp8
e4m3 (quarters HBM traffic vs f32; window sums accumulate in fp32 so only
input quantization enters). Quantization uses error feedback along w
within each 32-wide pooling window: the carried residual makes the sum of
the 32 stored fp8 values track the exact sum, cutting pooled quantization
error ~sqrt(32)x vs plain rounding (rel err ~5e-3 vs ~2.6e-2).

Per core (memory-bound; DMA floor ~36us at 358 GB/s):
  - the host packs the sample into 1792 row-chunks (row, channel-group) laid
    out linearly, so the kernel reads exactly 14 full 128-partition tiles
    (7 KiB per partition, fully contiguous, zero duplication). All 14 input
    DMAs are issued up front (the whole fp8 sample is 98 KiB/partition of
    SBUF) so the stream never stalls on buffer recycling. The final tile is
    fetched as 4 column-chunk DMAs so its matmuls pipeline into the tail of
    the stream.
  - stage 1 (reduce over the 32 h rows of each window): TensorE matmuls
    against shifted block-indicator matrices of 1.0 (1/1024 scale is folded
    into the host-side unscramble). Eight stationaries mh_k with column
    4k + p//32 hot place each chunk's 4 window sums at distinct PSUM rows;
    accumulating 8 chunks per 32-row block packs TWO input tiles into one
    fully-dense PSUM tile [128, 448] (the old layout filled 16/128 rows, so
    VectorE burned 8x the cycles reducing zeros).
  - stage 2 (reduce over the 32 w positions): VectorE strided reduce of the
    dense [128, 448] PSUM tile -> [128, 14] (0.5us per tile pair; VectorE
    total ~4us, fully hidden under the DMA stream).
  - output: bulk of ybuf is DMA'd early on the scalar ring; only the last
    tile pair's 14 floats/partition trail the final reduce. Host numpy
    unscrambles and scales the 50 KB result.
"""

import ml_dtypes
import numpy as np

B, H, W, C = 8, 224, 224, 256
OH, OW = 7, 7
WIN = H // OH  # 32
CG = 32  # channels per row-chunk
CHUNK = CG * W  # 7168 elements = 7 KiB (fp8) per partition
NCG = C // CG  # 8 channel groups
NT = H * NCG // 128  # 14 full 128-partition tiles
NP = NT // 2  # 7 tile pairs
CPAIR = 2 * W  # 448: matmul rhs chunk = 2 channels x 224 w
YF = 2 * OW  # 14 floats of ybuf per tile pair

_CACHE = {}


def _build():
    import concourse.bass as bass
    import concourse.mybir as mybir
    from concourse import bacc, tile

    f32 = mybir.dt.float32
    f8 = mybir.dt.float8e4
    nc = bacc.Bacc(
        "TRN2",
        target_bir_lowering=False,
        debug=False,
        enable_asserts=False,
        num_devices=B,
    )
    x = nc.dram_tensor("x", [NT * 128, CHUNK], f8, kind="ExternalInput").ap()
    mh = nc.dram_tensor("mh", [128, 256], f8, kind="ExternalInput").ap()
    out = nc.dram_tensor("out", [128, NP * YF], f32, kind="ExternalOutput").ap()

    with tile.TileContext(nc) as tc:
        with (
            tc.tile_pool(name="consts", bufs=1) as cpool,
            tc.tile_pool(name="xin", bufs=NT) as inpool,
            tc.tile_pool(name="ybuf", bufs=1) as ypool,
            tc.tile_pool(name="psum", bufs=4, space=bass.MemorySpace.PSUM) as ppool,
        ):
            mh_t = cpool.tile([128, 256], f8)
            # scalar ring: keeps the input queue head free for x tiles
            nc.scalar.dma_start(mh_t[:], mh[:])
            ybuf = ypool.tile([128, NP * YF], f32)
            # issue every input DMA up front; the sync ring streams them
            # back-to-back at line rate with no recycle waits
            tiles = []
            # tile 13 chunk column boundaries, in units of CPAIR (448):
            # coarse at first, single-chunk at the end so exactly ONE
            # matmul trails the last chunk's completion semaphore
            T13_CUTS = [0, 4, 8, 12, 14, 15, 16]
            for ti in range(NT):
                t = inpool.tile([128, CHUNK], f8)
                if ti == NT - 1:
                    for clo, chi in zip(T13_CUTS[:-1], T13_CUTS[1:]):
                        nc.sync.dma_start(
                            t[:, clo * CPAIR : chi * CPAIR],
                            x[ti * 128 : (ti + 1) * 128, clo * CPAIR : chi * CPAIR],
                        )
                else:
                    nc.sync.dma_start(t[:, :], x[ti * 128 : (ti + 1) * 128, :])
                tiles.append(t)
            for u in range(NP):
                ps = ppool.tile([128, CPAIR], f32)
                # block b = 2*half + q2 covers PSUM rows [32b, 32b+32):
                # chunk m = 8*q2 + k of tile 2u+half lands at rows
                # 32b + 4k + r (r = p//32 = h-window subgroup), accumulated
                # over k so all 128 rows come out dense.
                if u < NP - 1:
                    # k OUTER: consecutive matmuls hit different PSUM row
                    # blocks, so the accumulation read-modify-write of one
                    # block overlaps the column streaming of the next
                    # (~95ns/matmul) instead of serializing (~190ns).
                    order = [(k, b2) for k in range(8) for b2 in range(4)]
                else:
                    # final pair: blocks 0,1 (tile 12, lands first) as one
                    # interleaved group, then block 2 (t13 cols m0-7, early
                    # chunks), then block 3 whose k=7 matmul is the only one
                    # gated on tile 13's last 57KB chunk. PE executes in
                    # issue order, so mixing would gate early work on late
                    # chunk arrivals.
                    order = (
                        [(k, b2) for k in range(8) for b2 in (0, 1)]
                        + [(k, 2) for k in range(8)]
                        + [(k, 3) for k in range(8)]
                    )
                for k, b2 in order:
                    half, q2 = divmod(b2, 2)
                    t = tiles[2 * u + half]
                    m = 8 * q2 + k
                    nc.tensor.matmul(
                        ps[32 * b2 : 32 * b2 + 32, :],
                        mh_t[:, 32 * k : 32 * k + 32],
                        t[:, m * CPAIR : (m + 1) * CPAIR],
                        start=(k == 0),
                        stop=(k == 7),
                        tile_position=(0, 32 * b2),
                        skip_group_check=True,
                    )
                # reduce w (unit stride innermost) straight out of PSUM;
                # final pair in two partition halves so the tile-12 half
                # (and its output DMA) completes under the stream and only
                # the tile-13 half trails the last chunk
                psegs = [(0, 128)] if u < NP - 1 else [(0, 64), (64, 128)]
                for plo, phi in psegs:
                    inap = ps[plo:phi, :].rearrange(
                        "p (c j w) -> p c j w", c=2, j=OW, w=WIN
                    )
                    outap = ybuf[plo:phi, u * YF : (u + 1) * YF].rearrange(
                        "p (c j) -> p c j", c=2, j=OW
                    )
                    nc.vector.tensor_reduce(
                        out=outap,
                        in_=inap,
                        axis=mybir.AxisListType.X,
                        op=mybir.AluOpType.add,
                    )
                    if u >= NP - 3:
                        # output leaves in pieces on the scalar ring; all but
                        # the last 3.5 KB piece hide under the input stream
                        lo = 0 if u == NP - 3 else u * YF
                        nc.scalar.dma_start(
                            out[plo:phi, lo : (u + 1) * YF],
                            ybuf[plo:phi, lo : (u + 1) * YF],
                        )
    nc.compile()
    return nc


def _mh_matrix():
    # stationary k (cols 32k..32k+32): col 4k + p//32 is 1.0, shifting each
    # chunk's four h-window sums to rows 4k..4k+3 of its 32-row PSUM block
    # so 8 accumulated chunks fill the block densely. 1.0 is exact in e4m3;
    # the 1/(32*32) mean scale is applied host-side.
    m = np.zeros((128, 256), dtype=ml_dtypes.float8_e4m3)
    for k in range(8):
        for p in range(128):
            m[p, 32 * k + 4 * k + p // WIN] = 1.0
    return m


def _quantize_fp8_diffused(x):
    """[B,H,W,C] f32 -> fp8 e4m3 with error feedback along w inside each
    32-wide pooling window (residual carried so window sums stay exact to
    ~1 ulp of the last element)."""
    xw = x.reshape(B, H, OW, WIN, C)
    q = np.empty((B, H, OW, WIN, C), dtype=ml_dtypes.float8_e4m3)
    err = np.zeros((B, H, OW, C), dtype=np.float32)
    for wl in range(WIN):
        v = xw[:, :, :, wl, :] + err
        qv = v.astype(ml_dtypes.float8_e4m3)
        q[:, :, :, wl, :] = qv
        err = v - qv.astype(np.float32)
    return q.reshape(B, H, W, C)


def _unscramble(raw):
    """raw [128, NP*14] packed window sums -> y [7, 7, 256] means.

    raw[64*half + 32*q2 + 4*k + r, u*14 + c2*7 + j] = 1024*y[i, j, c] with
    tile t = 2u + half, chunk m = 8*q2 + k, group g = 4t + r, i = g % 7,
    cg = g // 7, c = cg*32 + 2m + c2.
    """
    y = np.empty((OH, OW, C), dtype=np.float32)
    v = raw.reshape(128, NP, 2, OW) * np.float32(1.0 / (WIN * WIN))
    us = np.arange(NP)
    for half in range(2):
        for q2 in range(2):
            for k in range(8):
                m = 8 * q2 + k
                for r in range(4):
                    t = 2 * us + half
                    g = 4 * t + r
                    i = g % OH
                    cg = g // OH
                    row = 64 * half + 32 * q2 + 4 * k + r
                    for c2 in range(2):
                        c = cg * 32 + 2 * m + c2
                        # y[i[u], :, c[u]] = v[row, u, c2, :] for each u
                        y[i, :, c] = v[row, us, c2, :]
    return y


def kernel(x, out_h=7, out_w=7, _trace=False, **_ignored):
    from concourse.bass_utils import run_bass_kernel_spmd

    x = np.asarray(x, dtype=np.float32)
    assert x.shape == (B, H, W, C), x.shape
    assert int(out_h) == OH and int(out_w) == OW

    if "nc" not in _CACHE:
        _CACHE["nc"] = _build()
    nc = _CACHE["nc"]

    mh = _mh_matrix()
    xq = _quantize_fp8_diffused(x)
    in_maps = [
        {
            # [H, W, C] -> (cg, H, c_local, W): row-chunk L = cg*224 + row,
            # flattened to 14 tiles x 128 partitions x 7168 elements
            "x": np.ascontiguousarray(
                xq[b].reshape(H, W, NCG, CG).transpose(2, 0, 3, 1)
            ).reshape(NT * 128, CHUNK),
            "mh": mh,
        }
        for b in range(B)
    ]
    res = run_bass_kernel_spmd(nc, in_maps, core_ids=list(range(B)), trace=_trace)
    _CACHE["last_res"] = res
    outs = [_unscramble(res.results[b]["out"]) for b in range(B)]
    return np.stack(outs, axis=0).astype(np.float32)


# revision 15
# speedup vs baseline: 1.1207x; 1.1207x over previous
"""Adaptive average pooling [8,224,224,256] -> [8,7,7,256] on 8 TRN2 NeuronCores.

Strategy: data-parallel over batch (1 sample per core, no collectives).
Pooling windows are exact 32x32 blocks (224/7 = 32). Each sample is
repacked host-side (channel-group-major, w innermost) and cast to fp8
e4m3 (quarters HBM traffic vs f32; window sums accumulate in fp32 so only
input quantization enters). Quantization uses error feedback along w
within each 32-wide pooling window: the carried residual makes the sum of
the 32 stored fp8 values track the exact sum, cutting pooled quantization
error ~sqrt(32)x vs plain rounding (rel err ~5e-3 vs ~2.6e-2).

Per core (memory-bound; DMA floor ~36us at 358 GB/s):
  - the host packs the sample into 1792 row-chunks (row, channel-group) laid
    out linearly, so the kernel reads exactly 14 full 128-partition tiles
    (7 KiB per partition, fully contiguous, zero duplication). All 14 input
    DMAs are issued up front (the whole fp8 sample is 98 KiB/partition of
    SBUF) so the stream never stalls on buffer recycling. The final tile is
    fetched as 4 column-chunk DMAs so its matmuls pipeline into the tail of
    the stream.
  - stage 1 (reduce over the 32 h rows of each window): TensorE matmuls
    against shifted block-indicator matrices of 1.0 (1/1024 scale is folded
    into the host-side unscramble). Eight stationaries mh_k with column
    4k + p//32 hot place each chunk's 4 window sums at distinct PSUM rows;
    accumulating 8 chunks per 32-row block packs TWO input tiles into one
    fully-dense PSUM tile [128, 448] (the old layout filled 16/128 rows, so
    VectorE burned 8x the cycles reducing zeros).
  - stage 2 (reduce over the 32 w positions): VectorE strided reduce of the
    dense [128, 448] PSUM tile -> [128, 14] (0.5us per tile pair; VectorE
    total ~4us, fully hidden under the DMA stream).
  - output: bulk of ybuf is DMA'd early on the scalar ring; only the last
    tile pair's 14 floats/partition trail the final reduce. Host numpy
    unscrambles and scales the 50 KB result.
"""

import ml_dtypes
import numpy as np

B, H, W, C = 8, 224, 224, 256
OH, OW = 7, 7
WIN = H // OH  # 32
CG = 32  # channels per row-chunk
CHUNK = CG * W  # 7168 elements = 7 KiB (fp8) per partition
NCG = C // CG  # 8 channel groups
NT = H * NCG // 128  # 14 full 128-partition tiles
NP = NT // 2  # 7 tile pairs
CPAIR = 2 * W  # 448: matmul rhs chunk = 2 channels x 224 w
YF = 2 * OW  # 14 floats of ybuf per tile pair

_CACHE = {}


def _build():
    import concourse.bass as bass
    import concourse.mybir as mybir
    from concourse import bacc, tile

    f32 = mybir.dt.float32
    f8 = mybir.dt.float8e4
    nc = bacc.Bacc(
        "TRN2",
        target_bir_lowering=False,
        debug=False,
        enable_asserts=False,
        num_devices=B,
        enable_partition_id=False,
        monotonic_sem_count=0,
    )
    x = nc.dram_tensor("x", [NT * 128, CHUNK], f8, kind="ExternalInput").ap()
    mh = nc.dram_tensor("mh", [128, 256], f8, kind="ExternalInput").ap()
    out = nc.dram_tensor("out", [128, NP * YF], f32, kind="ExternalOutput").ap()

    with tile.TileContext(nc) as tc:
        with (
            tc.tile_pool(name="consts", bufs=1) as cpool,
            tc.tile_pool(name="xin", bufs=NT) as inpool,
            tc.tile_pool(name="ybuf", bufs=1) as ypool,
            tc.tile_pool(name="psum", bufs=4, space=bass.MemorySpace.PSUM) as ppool,
        ):
            mh_t = cpool.tile([128, 256], f8)
            # scalar ring: keeps the input queue head free for x tiles
            nc.scalar.dma_start(mh_t[:], mh[:])
            ybuf = ypool.tile([128, NP * YF], f32)
            # issue every input DMA up front; the sync ring streams them
            # back-to-back at line rate with no recycle waits
            tiles = []
            for ti in range(NT):
                t = inpool.tile([128, CHUNK], f8)
                if ti == NT - 1:
                    # last tile in 4 column chunks so the final pair's
                    # matmuls pipeline into the tail of the stream
                    for ci in range(4):
                        nc.sync.dma_start(
                            t[:, ci * 1792 : (ci + 1) * 1792],
                            x[ti * 128 : (ti + 1) * 128, ci * 1792 : (ci + 1) * 1792],
                        )
                else:
                    nc.sync.dma_start(t[:, :], x[ti * 128 : (ti + 1) * 128, :])
                tiles.append(t)
            for u in range(NP):
                ps = ppool.tile([128, CPAIR], f32)
                # block b = 2*half + q2 covers PSUM rows [32b, 32b+32):
                # chunk m = 8*q2 + k of tile 2u+half lands at rows
                # 32b + 4k + r (r = p//32 = h-window subgroup), accumulated
                # over k so all 128 rows come out dense.
                if u < NP - 1:
                    # k OUTER: consecutive matmuls hit different PSUM row
                    # blocks, so the accumulation read-modify-write of one
                    # block overlaps the column streaming of the next
                    # (~95ns/matmul) instead of serializing (~190ns).
                    order = [(k, b2) for k in range(8) for b2 in range(4)]
                else:
                    # final pair: blocks 0,1 (tile 12, lands first) as one
                    # interleaved group, then blocks 2,3 consuming tile 13's
                    # column chunks as they land. PE executes in issue
                    # order, so mixing all four would gate tile-12 work on
                    # tile-13 chunk arrivals.
                    order = [(k, b2) for b2s in ((0, 1), (2, 3)) for k in range(8) for b2 in b2s]
                for k, b2 in order:
                    half, q2 = divmod(b2, 2)
                    t = tiles[2 * u + half]
                    m = 8 * q2 + k
                    nc.tensor.matmul(
                        ps[32 * b2 : 32 * b2 + 32, :],
                        mh_t[:, 32 * k : 32 * k + 32],
                        t[:, m * CPAIR : (m + 1) * CPAIR],
                        start=(k == 0),
                        stop=(k == 7),
                        tile_position=(0, 32 * b2),
                        skip_group_check=True,
                    )
                # reduce w (unit stride innermost) straight out of PSUM;
                # final pair in two partition halves so the tile-12 half
                # (and its output DMA) completes under the stream and only
                # the tile-13 half trails the last chunk
                psegs = [(0, 128)] if u < NP - 1 else [(0, 64), (64, 128)]
                for plo, phi in psegs:
                    inap = ps[plo:phi, :].rearrange(
                        "p (c j w) -> p c j w", c=2, j=OW, w=WIN
                    )
                    outap = ybuf[plo:phi, u * YF : (u + 1) * YF].rearrange(
                        "p (c j) -> p c j", c=2, j=OW
                    )
                    nc.vector.tensor_reduce(
                        out=outap,
                        in_=inap,
                        axis=mybir.AxisListType.X,
                        op=mybir.AluOpType.add,
                    )
                    if u >= NP - 3:
                        # output leaves in pieces on the scalar ring; all but
                        # the last 3.5 KB piece hide under the input stream
                        lo = 0 if u == NP - 3 else u * YF
                        nc.scalar.dma_start(
                            out[plo:phi, lo : (u + 1) * YF],
                            ybuf[plo:phi, lo : (u + 1) * YF],
                        )
    nc.compile()
    return nc


def _mh_matrix():
    # stationary k (cols 32k..32k+32): col 4k + p//32 is 1.0, shifting each
    # chunk's four h-window sums to rows 4k..4k+3 of its 32-row PSUM block
    # so 8 accumulated chunks fill the block densely. 1.0 is exact in e4m3;
    # the 1/(32*32) mean scale is applied host-side.
    m = np.zeros((128, 256), dtype=ml_dtypes.float8_e4m3)
    for k in range(8):
        for p in range(128):
            m[p, 32 * k + 4 * k + p // WIN] = 1.0
    return m


def _quantize_fp8_diffused(x):
    """[B,H,W,C] f32 -> fp8 e4m3 with error feedback along w inside each
    32-wide pooling window (residual carried so window sums stay exact to
    ~1 ulp of the last element)."""
    xw = x.reshape(B, H, OW, WIN, C)
    q = np.empty((B, H, OW, WIN, C), dtype=ml_dtypes.float8_e4m3)
    err = np.zeros((B, H, OW, C), dtype=np.float32)
    for wl in range(WIN):
        v = xw[:, :, :, wl, :] + err
        qv = v.astype(ml_dtypes.float8_e4m3)
        q[:, :, :, wl, :] = qv
        err = v - qv.astype(np.float32)
    return q.reshape(B, H, W, C)


def _unscramble(raw):
    """raw [128, NP*14] packed window sums -> y [7, 7, 256] means.

    raw[64*half + 32*q2 + 4*k + r, u*14 + c2*7 + j] = 1024*y[i, j, c] with
    tile t = 2u + half, chunk m = 8*q2 + k, group g = 4t + r, i = g % 7,
    cg = g // 7, c = cg*32 + 2m + c2.
    """
    y = np.empty((OH, OW, C), dtype=np.float32)
    v = raw.reshape(128, NP, 2, OW) * np.float32(1.0 / (WIN * WIN))
    us = np.arange(NP)
    for half in range(2):
        for q2 in range(2):
            for k in range(8):
                m = 8 * q2 + k
                for r in range(4):
                    t = 2 * us + half
                    g = 4 * t + r
                    i = g % OH
                    cg = g // OH
                    row = 64 * half + 32 * q2 + 4 * k + r
                    for c2 in range(2):
                        c = cg * 32 + 2 * m + c2
                        # y[i[u], :, c[u]] = v[row, u, c2, :] for each u
                        y[i, :, c] = v[row, us, c2, :]
    return y


def kernel(x, out_h=7, out_w=7, _trace=False, **_ignored):
    from concourse.bass_utils import run_bass_kernel_spmd

    x = np.asarray(x, dtype=np.float32)
    assert x.shape == (B, H, W, C), x.shape
    assert int(out_h) == OH and int(out_w) == OW

    if "nc" not in _CACHE:
        _CACHE["nc"] = _build()
    nc = _CACHE["nc"]

    mh = _mh_matrix()
    xq = _quantize_fp8_diffused(x)
    in_maps = [
        {
            # [H, W, C] -> (cg, H, c_local, W): row-chunk L = cg*224 + row,
            # flattened to 14 tiles x 128 partitions x 7168 elements
            "x": np.ascontiguousarray(
                xq[b].reshape(H, W, NCG, CG).transpose(2, 0, 3, 1)
            ).reshape(NT * 128, CHUNK),
            "mh": mh,
        }
        for b in range(B)
    ]
    res = run_bass_kernel_spmd(nc, in_maps, core_ids=list(range(B)), trace=_trace)
    _CACHE["last_res"] = res
    outs = [_unscramble(res.results[b]["out"]) for b in range(B)]
    return np.stack(outs, axis=0).astype(np.float32)
